# revision 54
# baseline (speedup 1.0000x reference)
"""Trainium2 Bass kernel for the non-local-block module (nn_CNL_747324309589).

Sharding: data-parallel over batch — 16 batches across 8 NeuronCores, 2 per
core, no collectives.  Per batch (dims: HIGH=2048, LOW=512, N=H*W=1152):

    theta_xT[n,c] = sum_h xh[h,n]·thwT[h,c] + thb[c]      (x_h chunks = lhsT)
    phi_xT [n,d]  = sum_l xl[l,n]·phwT[l,d] + phb[d]      (phw,phb pre-scaled by 1/512)
    g_x    [d,n]  = sum_l gwT[l,d]·xl[l,n]  + gb[d]
    attT   [d,c]  = sum_n phi_xT[n,d]·theta_xT[n,c]       (= energy^T/512)
    y      [c,n]  = sum_d attT[d,c]·g_x[d,n]
    w_y    [o,n]  = sum_c wwT[c,o]·y[c,n]                 (BN scale pre-folded into ww)
    out    [o,n]  = w_y + bnt[o] + xh[o,n]                (shift + residual in one DVE op)

All matmul operands are bf16 (same 1 row/cycle PE rate as float32r in the
cost model, half the DMA bytes and SBUF footprint), accumulating fp32 in
PSUM; the output is DMA'd out as bf16 and widened to fp32 on the host.
Weights are loaded once (not per batch).  theta's first 512 contraction
channels additionally run as fp8e4m3 DoubleRow matmuls (0.5 cycles/row —
saves ~7.7us of PE time for ~1.7% relative error, within the 2e-2 budget).
theta accumulates k-major across 6 PSUM banks so each (thw quarter, x_h
chunk) pair is consumed as it lands, with m=6..8 trailing m-major to cover
the drain latency; phase order A2(phi), A1(theta), A3(g) puts g between
theta's drain burst and B1.  PE warmup matmuls burn the p-state ramp during
the prologue; DMA issue is spread across the SP, ACT and Pool queues (each
dma_start serializes ~1.2us on its queue) so the theta input stream keeps
ahead of the fp8-accelerated consumption.  Batch b+1's x_l / x_h prefetch
DMAs issue from the otherwise idle ACT queue inside batch b's C loop.
"""

import numpy as np

import concourse.bass as bass
import concourse.bacc as bacc
import concourse.mybir as mybir
import concourse.tile as tile
from concourse.bass import ts

B, HIGH, LOW, H, W = 16, 2048, 512, 48, 24
N = H * W            # 1152
NCORES = 8
BPC = B // NCORES    # 2 batches per core
P = 128
KH = HIGH // P       # 16
KL = LOW // P        # 4
MN = N // P          # 9
NSPLIT = 3
NW = N // NSPLIT     # 384
BN_EPS = 1e-5

F32 = mybir.dt.float32
BF16 = mybir.dt.bfloat16
FP8 = mybir.dt.float8e4
ADD = mybir.AluOpType.add
MULT = mybir.AluOpType.mult
AF = mybir.ActivationFunctionType
DR = mybir.MatmulPerfMode.DoubleRow

# theta hybrid: first KF8=512 of 2048 contraction channels run as fp8e4m3
# DoubleRow matmuls (2x PE rate), the rest bf16.  Host-side scales make the
# shared-PSUM accumulation exact: fp8 operands carry 8x (x_h) and 256x
# (theta_w); the bf16 theta weights carry the matching 2048x, and the drain
# multiplies by 2^-11.  Quantization error of the fp8 fraction adds ~1.7%
# relative error on the output, within the 2e-2 budget.
XH8_SCALE = 8.0
THW8_SCALE = 256.0
TH_SCALE = XH8_SCALE * THW8_SCALE  # 2^11
NG8 = 2               # DoubleRow pair-groups (256 channels each)
KF8 = NG8 * 256       # fp8 channels
KB0 = KF8 // P        # first bf16 k-chunk (4)


def _build_module() -> bass.Bass:
    nc = bacc.Bacc()
    x_h = nc.dram_tensor("x_h", [BPC, HIGH, N], BF16, kind="ExternalInput")
    x_l = nc.dram_tensor("x_l", [BPC, LOW, N], BF16, kind="ExternalInput")
    thw = nc.dram_tensor("thw", [P, KH, LOW], BF16, kind="ExternalInput")
    thw8 = nc.dram_tensor("thw8", [P, NG8, 2, LOW], FP8, kind="ExternalInput")
    xh8 = nc.dram_tensor("xh8", [BPC, P, NG8, 2, N], FP8, kind="ExternalInput")
    phw = nc.dram_tensor("phw", [P, KL, LOW], BF16, kind="ExternalInput")
    gw = nc.dram_tensor("gw", [P, KL, LOW], BF16, kind="ExternalInput")
    ww = nc.dram_tensor("ww", [P, KL, HIGH], BF16, kind="ExternalInput")
    thpb = nc.dram_tensor("thpb", [1, 2 * LOW], BF16, kind="ExternalInput")
    gbnt = nc.dram_tensor("gbnt", [P, KL + KH], F32, kind="ExternalInput")
    out = nc.dram_tensor("out", [BPC, HIGH, N], BF16, kind="ExternalOutput")
    thp_dram = nc.dram_tensor("thp_dram", [64, MN, LOW], BF16, kind="Internal")

    with tile.TileContext(nc) as tc:
        with (
            tc.tile_pool(name="consts", bufs=1) as cpool,
            tc.tile_pool(name="xh", bufs=KH) as xhpool,
            tc.tile_pool(name="xh8", bufs=2) as xh8pool,
            tc.tile_pool(name="xl", bufs=1) as xlpool,
            tc.tile_pool(name="att", bufs=1) as attpool,
            tc.tile_pool(name="mid", bufs=1) as midpool,
            tc.tile_pool(name="stg", bufs=3) as stgpool,
            tc.tile_pool(name="psum", bufs=8, space="PSUM") as pspool,
        ):
            # PE warmup: the p-state ramp (0.65/1.2 GHz for the first ~3us of
            # PE activity) burns on throwaway matmuls while the first DMAs
            # land, so real matmuls start at the full 2.4 GHz clock
            wu = cpool.tile([P, 64], BF16, tag="wu")
            nc.gpsimd.memset(wu[:], 0.0)
            wps = pspool.tile([P, 512], F32, tag="ps", name="wps")
            for i in range(36):
                nc.tensor.matmul(
                    wps[:64, :64], wu[:], wu[:], start=True, stop=True
                )

            # A2's inputs go first so phi can start ASAP.  Each dma_start
            # costs ~1.2us serialized on its issuing queue, so the three
            # first-dependency transfers are spread across SP, Pool (SWDGE)
            # and ACT so their issue latencies overlap.
            phw_sb = cpool.tile([P, KL, LOW], BF16, tag="phw")
            nc.sync.dma_start(phw_sb[:, :1], phw[:, :1])
            nc.sync.dma_start(phw_sb[:, 1:2], phw[:, 1:2])
            nc.gpsimd.dma_start(phw_sb[:, 2:], phw[:, 2:])
            xl0_sb = xlpool.tile([P, KL, N], BF16, tag="xl")
            xl0_r = x_l[0].rearrange("(ko p) n -> p ko n", p=P)
            # m=0 sub-chunk first so A2's first group starts on ~100KB
            nc.scalar.dma_start(xl0_sb[:, :, :P], xl0_r[:, :, :P])
            nc.scalar.dma_start(xl0_sb[:, :, P:NW], xl0_r[:, :, P:NW])
            for nn in range(1, NSPLIT):
                nc.scalar.dma_start(
                    xl0_sb[:, :, ts(nn, NW)], xl0_r[:, :, ts(nn, NW)]
                )
            thpb_sb = cpool.tile([P, 2 * LOW], BF16, tag="thpb")
            nc.sync.dma_start(thpb_sb[:], thpb[:].to_broadcast((P, 2 * LOW)))
            thb_sb = thpb_sb[:, :LOW]
            phb_sb = thpb_sb[:, LOW:]
            # theta fp8 pair tensors early — the n-half1 partial DR groups
            # interleaved into A2 consume them from ~6us; xh8 split per
            # pair-group so g=0 quadrants can start on the first half
            thw8_sb = cpool.tile([P, NG8, 2, LOW], FP8, tag="thw8")
            nc.sync.dma_start(thw8_sb[:], thw8[:])
            xh8_sb = xh8pool.tile([P, NG8, 2, N], FP8, tag="xh8")
            nc.sync.dma_start(xh8_sb[:, 0], xh8[0, :, 0])
            nc.sync.dma_start(xh8_sb[:, 1], xh8[0, :, 1])
            gbnt_sb = cpool.tile([P, KL + KH], F32, tag="gbnt")
            nc.sync.dma_start(gbnt_sb[:], gbnt[:])
            gb_sb = gbnt_sb[:, :KL]
            bnt_sb = gbnt_sb[:, KL:]
            # bf16 theta inputs in A1's k-major consumption order (k=4..15);
            # the stream is spread across all three issue queues (thw
            # quarters on Pool, xh chunks alternating SP/ACT) so it keeps
            # ahead of the fp8-accelerated A1 consumption; xh chunks 0..3
            # (only needed for C's residual) go last
            thw_sb = cpool.tile([P, KH, LOW], BF16, tag="thw")
            xh_t: list = [None] * KH
            for q in range(1, 4):
                nc.gpsimd.dma_start(
                    thw_sb[:, ts(q, KH // 4), :], thw[:, ts(q, KH // 4), :]
                )
                for k in range(q * 4, q * 4 + 4):
                    t_ = xhpool.tile([P, N], BF16, tag="xh", name=f"xh_{k}")
                    eng = nc.sync if k % 2 == 0 else nc.scalar
                    eng.dma_start(t_[:], x_h[0, ts(k, P), :])
                    xh_t[k] = t_
            for k in range(KB0):
                t_ = xhpool.tile([P, N], BF16, tag="xh", name=f"xh_{k}")
                eng = nc.sync if k % 2 == 0 else nc.scalar
                eng.dma_start(t_[:], x_h[0, ts(k, P), :])
                xh_t[k] = t_
            gw_sb = cpool.tile([P, KL, LOW], BF16, tag="gw")
            nc.scalar.dma_start(gw_sb[:], gw[:])
            ww_sb = cpool.tile([P, KL, HIGH], BF16, tag="ww")
            for k in range(KL):
                nc.sync.dma_start(ww_sb[:, k], ww[:, k])

            for b in range(BPC):
                if b > 0:
                    xl_sb = xl_next
                    xh_t = xh_next
                    xh8_sb = xh8_next
                else:
                    xl_sb = xl0_sb

                # phi_xT [n, d] (phase A2), with theta's n-half1 fp8 partials
                # (tiny DR groups) interleaved so their PSUM allocations ride
                # A2's progressive bank frees
                ph_sb = midpool.tile([P, MN, LOW], BF16, tag="ph")
                th_sb = midpool.tile([P, MN, LOW], BF16, tag="th")
                thp_sb = midpool.tile([64, MN, LOW], BF16, tag="thp")
                thp2_sb = midpool.tile([P, MN, LOW], BF16, tag="thp2")

                def a1_dr(ps_m, m, half, start, stop):
                    for g in range(NG8):
                        for ch in range(2):
                            nc.tensor.matmul(
                                ps_m[0:64, ts(ch, 256)],
                                xh8_sb[:, g, :, m * P + half * 64 : m * P + half * 64 + 64],
                                thw8_sb[:, g, :, ts(ch, 256)],
                                start=start and g == 0 and ch == 0,
                                stop=stop and g == NG8 - 1 and ch == 1,
                                perf_mode=DR,
                                skip_group_check=True,
                            )

                def a1_partial(mp):
                    psp = pspool.tile([P, 512], F32, tag="ps", name=f"ps_p_{mp}")
                    a1_dr(psp, mp, 1, True, True)
                    nc.vector.scalar_tensor_tensor(
                        thp_sb[:, mp, :], psp[0:64, :], 1.0 / TH_SCALE,
                        thb_sb[0:64, :], MULT, ADD,
                    )

                for m in range(MN):
                    ps = pspool.tile([P, 512], F32, tag="ps")
                    for k in range(KL):
                        nc.tensor.matmul(
                            ps[:],
                            xl_sb[:, k, ts(m, P)],
                            phw_sb[:, k, :],
                            start=(k == 0),
                            stop=(k == KL - 1),
                        )
                    nc.vector.tensor_tensor(ph_sb[:, m, :], ps[:], phb_sb[:], ADD)
                    # partials deferred 4 groups so their fp8 inputs land
                    if m >= 4:
                        a1_partial(m - 4)
                for mp in range(MN - 4, MN):
                    a1_partial(mp)
                nc.sync.dma_start(thp_dram[:], thp_sb[:])
                nc.sync.dma_start(thp2_sb[64:128], thp_dram[:])

                # theta_xT [n, c] (phase A1).  Channels 0..255 run as fp8
                # DoubleRow (2x PE rate); DR outputs can only land on PSUM
                # partitions 0..63.  Each main group opens with its n-half0
                # DR quadrants (ch0 start=True arms the 2KB zero region,
                # auto-zeroing partitions 64..127 for the bf16 k=2 matmul),
                # then accumulates bf16 k-major over 8 banks so chunk k is
                # consumed right after it lands; m=8 runs m-major.  The
                # n-half1 partials were computed during A2 and remapped to
                # partitions 64..127; the drains combine them.
                def a1_drain(ps_m, m):
                    nc.vector.scalar_tensor_tensor(
                        th_sb[0:64, m, :], ps_m[0:64, :], 1.0 / TH_SCALE,
                        thb_sb[0:64, :], MULT, ADD,
                    )
                    nc.vector.scalar_tensor_tensor(
                        th_sb[64:128, m, :], ps_m[64:128, :], 1.0 / TH_SCALE,
                        thp2_sb[64:128, m, :], MULT, ADD,
                    )

                ps_a1 = [
                    pspool.tile([P, 512], F32, tag="ps", name=f"ps_a1_{m}")
                    for m in range(6)
                ]
                for m in range(6):
                    # k=2 opens the group full-width (start zeroes the whole
                    # bank incl. partitions 64..127), then the DR quadrants
                    # accumulate onto partitions 0..63
                    nc.tensor.matmul(
                        ps_a1[m][:],
                        xh_t[KB0][:, ts(m, P)],
                        thw_sb[:, KB0, :],
                        start=True,
                        stop=False,
                        skip_group_check=True,
                    )
                    a1_dr(ps_a1[m], m, 0, False, False)
                for k in range(KB0 + 1, KH):
                    for m in range(6):
                        nc.tensor.matmul(
                            ps_a1[m][:],
                            xh_t[k][:, ts(m, P)],
                            thw_sb[:, k, :],
                            start=False,
                            stop=(k == KH - 1),
                            skip_group_check=True,
                        )
                for m in range(6):
                    a1_drain(ps_a1[m], m)
                # m=6,7 run in the spare banks, covering the drain latency;
                # m=8 finishes m-major
                for m in (6, 7, 8):
                    ps = pspool.tile([P, 512], F32, tag="ps", name=f"ps_t_{m}")
                    nc.tensor.matmul(
                        ps[:], xh_t[KB0][:, ts(m, P)], thw_sb[:, KB0, :],
                        start=True, stop=False, skip_group_check=True,
                    )
                    a1_dr(ps, m, 0, False, False)
                    for k in range(KB0 + 1, KH):
                        nc.tensor.matmul(
                            ps[:],
                            xh_t[k][:, ts(m, P)],
                            thw_sb[:, k, :],
                            start=False,
                            stop=(k == KH - 1),
                            skip_group_check=True,
                        )
                    a1_drain(ps, m)

                # g_x [d, n] (phase A3) — sits between theta's drain burst
                # and B1 so the th drains overlap PE work
                g_sb = midpool.tile([P, KL, N], BF16, tag="g")
                for md in range(KL):
                    for nn in range(NSPLIT):
                        ps = pspool.tile([P, 512], F32, tag="ps")
                        for k in range(KL):
                            nc.tensor.matmul(
                                ps[:, :NW],
                                gw_sb[:, k, ts(md, P)],
                                xl_sb[:, k, ts(nn, NW)],
                                start=(k == 0),
                                stop=(k == KL - 1),
                            )
                        nc.scalar.activation(
                            g_sb[:, md, ts(nn, NW)],
                            ps[:, :NW],
                            AF.Identity,
                            bias=gb_sb[:, md : md + 1],
                        )

                # batch b+1 x_l prefetch: WAR on this batch's A2/A3 reads,
                # issued from the ACT queue (behind A3's drains)
                if b + 1 < BPC:
                    xl_next = xlpool.tile([P, KL, N], BF16, tag="xl")
                    xl1_r = x_l[b + 1].rearrange("(ko p) n -> p ko n", p=P)
                    for nn in range(NSPLIT):
                        nc.scalar.dma_start(
                            xl_next[:, :, ts(nn, NW)], xl1_r[:, :, ts(nn, NW)]
                        )

                # attT [d, c] = energy^T/512 (phase B1)
                att_sb = attpool.tile([P, KL, LOW], BF16, tag="att")
                for md in range(KL):
                    ps = pspool.tile([P, 512], F32, tag="ps")
                    for k in range(MN):
                        nc.tensor.matmul(
                            ps[:],
                            ph_sb[:, k, ts(md, P)],
                            th_sb[:, k, :],
                            start=(k == 0),
                            stop=(k == MN - 1),
                        )
                    nc.scalar.activation(att_sb[:, md, :], ps[:], AF.Identity)

                # y [c, n] (phase B2); y shares the theta_xT slot
                y_sb = midpool.tile([P, KL, N], BF16, tag="th")
                for mc in range(KL):
                    for nn in range(NSPLIT):
                        ps = pspool.tile([P, 512], F32, tag="ps")
                        for k in range(KL):
                            nc.tensor.matmul(
                                ps[:, :NW],
                                att_sb[:, k, ts(mc, P)],
                                g_sb[:, k, ts(nn, NW)],
                                start=(k == 0),
                                stop=(k == KL - 1),
                            )
                        nc.scalar.activation(
                            y_sb[:, mc, ts(nn, NW)], ps[:, :NW], AF.Identity
                        )

                # w_y + BN + residual (phase C); output staged per mo stripe
                # and written as one DMA; batch b+1 x_h chunk prefetch issues
                # from ACT right after chunk mo's last read
                for mo in range(KH):
                    xt = xh_t[mo]
                    stg = stgpool.tile([P, N], BF16, tag="stg")
                    last = b == BPC - 1 and mo == KH - 1
                    for nn in range(NSPLIT):
                        ps = pspool.tile([P, 512], F32, tag="ps")
                        for k in range(KL):
                            nc.tensor.matmul(
                                ps[:, :NW],
                                ww_sb[:, k, ts(mo, P)],
                                y_sb[:, k, ts(nn, NW)],
                                start=(k == 0),
                                stop=(k == KL - 1),
                            )
                        nc.vector.scalar_tensor_tensor(
                            stg[:, ts(nn, NW)],
                            ps[:, :NW],
                            bnt_sb[:, mo : mo + 1],
                            xt[:, ts(nn, NW)],
                            ADD,
                            ADD,
                        )
                        # the very last stripe writes out in two DMAs so the
                        # final transfer after the last matmul is small
                        if last and nn == NSPLIT - 2:
                            nc.sync.dma_start(
                                out[b, ts(mo, P), : 2 * NW], stg[:, : 2 * NW]
                            )
                        elif last and nn == NSPLIT - 1:
                            nc.sync.dma_start(
                                out[b, ts(mo, P), 2 * NW :], stg[:, 2 * NW :]
                            )
                    if not last:
                        nc.sync.dma_start(out[b, ts(mo, P), :], stg[:])
                    if b + 1 < BPC:
                        if mo == 0:
                            xh_next = [None] * KH
                            xh8_next = xh8pool.tile([P, NG8, 2, N], FP8, tag="xh8")
                            nc.scalar.dma_start(xh8_next[:], xh8[b + 1])
                        t_ = xhpool.tile([P, N], BF16, tag="xh")
                        nc.scalar.dma_start(t_[:], x_h[b + 1, ts(mo, P), :])
                        xh_next[mo] = t_
    nc.compile()
    return nc


_CACHE: dict = {}


def _get_module() -> bass.Bass:
    if "nc" not in _CACHE:
        _CACHE["nc"] = _build_module()
    return _CACHE["nc"]


def _prep_maps(inputs: dict) -> list[dict]:
    import ml_dtypes

    BF = ml_dtypes.bfloat16
    f = lambda a: np.ascontiguousarray(np.asarray(a, dtype=np.float32))
    bf = lambda a: np.ascontiguousarray(np.asarray(a, dtype=np.float32).astype(BF))
    x_h = bf(inputs["x_h"]).reshape(B, HIGH, N)
    x_l = bf(inputs["x_l"]).reshape(B, LOW, N)
    theta_w = f(inputs["theta_w"])
    phi_w = f(inputs["phi_w"])
    g_w = f(inputs["g_w"])
    w_w = f(inputs["w_w"])

    # bf16 theta weights pre-scaled by 2^11 to match the fp8 partial's scale
    thw_h = (theta_w.T * np.float32(TH_SCALE)).reshape(KH, P, LOW) \
        .transpose(1, 0, 2).astype(BF)
    # fp8 pair tensors for the DoubleRow channels (0..255): pair index i
    # holds channels i*128+p
    F8 = ml_dtypes.float8_e4m3
    clip8 = lambda a: np.clip(a, -224.0, 224.0).astype(F8)
    # thw8[p, g, i, c] = theta_w[c, g*256 + i*128 + p] * 256
    thw8_h = np.ascontiguousarray(
        clip8(
            (theta_w[:, :KF8] * np.float32(THW8_SCALE)).T
            .reshape(NG8, 2, P, LOW).transpose(2, 0, 1, 3)
        )
    )
    # xh8[b, p, g, i, n] = x_h[b, g*256 + i*128 + p, n] * 8
    x_h32 = f(inputs["x_h"]).reshape(B, HIGH, N)
    xh8_h = np.ascontiguousarray(
        clip8(
            (x_h32[:, :KF8, :] * np.float32(XH8_SCALE))
            .reshape(B, NG8, 2, P, N).transpose(0, 3, 1, 2, 4)
        )
    )
    phw_h = (phi_w.T / np.float32(LOW)).reshape(KL, P, LOW).transpose(1, 0, 2).astype(BF)
    gw_h = g_w.T.reshape(KL, P, LOW).transpose(1, 0, 2).astype(BF)
    s = f(inputs["bn_gamma"]) / np.sqrt(f(inputs["bn_var"]) + np.float32(BN_EPS))
    # BN scale folded into the w conv weights; only the shift remains on-device
    ww_h = (w_w * s[:, None]).astype(np.float32).T.reshape(KL, P, HIGH) \
        .transpose(1, 0, 2).astype(BF)

    thpb_h = np.concatenate(
        [f(inputs["theta_b"]), f(inputs["phi_b"]) / np.float32(LOW)]
    ).reshape(1, 2 * LOW).astype(BF)
    gb_h = np.ascontiguousarray(f(inputs["g_b"]).reshape(KL, P).T)
    t = (f(inputs["w_b"]) - f(inputs["bn_mean"])) * s + f(inputs["bn_beta"])
    bnt_h = np.ascontiguousarray(t.astype(np.float32).reshape(KH, P).T)
    gbnt_h = np.ascontiguousarray(np.concatenate([gb_h, bnt_h], axis=1))

    shared = dict(
        thw=np.ascontiguousarray(thw_h),
        thw8=thw8_h,
        phw=np.ascontiguousarray(phw_h),
        gw=np.ascontiguousarray(gw_h),
        ww=np.ascontiguousarray(ww_h),
        thpb=thpb_h,
        gbnt=gbnt_h,
    )
    maps = []
    for c in range(NCORES):
        m = dict(shared)
        m["x_h"] = np.ascontiguousarray(x_h[c * BPC : (c + 1) * BPC])
        m["xh8"] = np.ascontiguousarray(xh8_h[c * BPC : (c + 1) * BPC])
        m["x_l"] = np.ascontiguousarray(x_l[c * BPC : (c + 1) * BPC])
        maps.append(m)
    return maps


def _run(inputs: dict, **kwargs):
    from concourse.bass_utils import run_bass_kernel_spmd

    nc = _get_module()
    in_maps = _prep_maps(inputs)
    res = run_bass_kernel_spmd(nc, in_maps, core_ids=list(range(NCORES)), **kwargs)
    parts = [np.asarray(r["out"], dtype=np.float32) for r in res.results]
    full = np.concatenate(parts, axis=0).reshape(B, HIGH, H, W)
    return full, res


def kernel(**inputs) -> np.ndarray:
    full, _ = _run(inputs)
    return full


# revision 55
# speedup vs baseline: 1.0068x; 1.0068x over previous
"""Trainium2 Bass kernel for the non-local-block module (nn_CNL_747324309589).

Sharding: data-parallel over batch — 16 batches across 8 NeuronCores, 2 per
core, no collectives.  Per batch (dims: HIGH=2048, LOW=512, N=H*W=1152):

    theta_xT[n,c] = sum_h xh[h,n]·thwT[h,c] + thb[c]      (x_h chunks = lhsT)
    phi_xT [n,d]  = sum_l xl[l,n]·phwT[l,d] + phb[d]      (phw,phb pre-scaled by 1/512)
    g_x    [d,n]  = sum_l gwT[l,d]·xl[l,n]  + gb[d]
    attT   [d,c]  = sum_n phi_xT[n,d]·theta_xT[n,c]       (= energy^T/512)
    y      [c,n]  = sum_d attT[d,c]·g_x[d,n]
    w_y    [o,n]  = sum_c wwT[c,o]·y[c,n]                 (BN scale pre-folded into ww)
    out    [o,n]  = w_y + bnt[o] + xh[o,n]                (shift + residual in one DVE op)

All matmul operands are bf16 (same 1 row/cycle PE rate as float32r in the
cost model, half the DMA bytes and SBUF footprint), accumulating fp32 in
PSUM; the output is DMA'd out as bf16 and widened to fp32 on the host.
Weights are loaded once (not per batch).  theta's first 512 contraction
channels additionally run as fp8e4m3 DoubleRow matmuls (0.5 cycles/row —
saves ~7.7us of PE time for ~1.7% relative error, within the 2e-2 budget).
theta accumulates k-major across 6 PSUM banks so each (thw quarter, x_h
chunk) pair is consumed as it lands, with m=6..8 trailing m-major to cover
the drain latency; phase order A2(phi), A1(theta), A3(g) puts g between
theta's drain burst and B1.  PE warmup matmuls burn the p-state ramp during
the prologue; DMA issue is spread across the SP, ACT and Pool queues (each
dma_start serializes ~1.2us on its queue) so the theta input stream keeps
ahead of the fp8-accelerated consumption.  Batch b+1's x_l / x_h prefetch
DMAs issue from the otherwise idle ACT queue inside batch b's C loop.
"""

import numpy as np

import concourse.bass as bass
import concourse.bacc as bacc
import concourse.mybir as mybir
import concourse.tile as tile
from concourse.bass import ts

B, HIGH, LOW, H, W = 16, 2048, 512, 48, 24
N = H * W            # 1152
NCORES = 8
BPC = B // NCORES    # 2 batches per core
P = 128
KH = HIGH // P       # 16
KL = LOW // P        # 4
MN = N // P          # 9
NSPLIT = 3
NW = N // NSPLIT     # 384
BN_EPS = 1e-5

F32 = mybir.dt.float32
BF16 = mybir.dt.bfloat16
FP8 = mybir.dt.float8e4
ADD = mybir.AluOpType.add
MULT = mybir.AluOpType.mult
AF = mybir.ActivationFunctionType
DR = mybir.MatmulPerfMode.DoubleRow

# theta hybrid: first KF8=512 of 2048 contraction channels run as fp8e4m3
# DoubleRow matmuls (2x PE rate), the rest bf16.  Host-side scales make the
# shared-PSUM accumulation exact: fp8 operands carry 8x (x_h) and 256x
# (theta_w); the bf16 theta weights carry the matching 2048x, and the drain
# multiplies by 2^-11.  Quantization error of the fp8 fraction adds ~1.7%
# relative error on the output, within the 2e-2 budget.
XH8_SCALE = 8.0
THW8_SCALE = 256.0
TH_SCALE = XH8_SCALE * THW8_SCALE  # 2^11
NG8 = 2               # DoubleRow pair-groups (256 channels each)
KF8 = NG8 * 256       # fp8 channels
KB0 = KF8 // P        # first bf16 k-chunk (4)


def _build_module() -> bass.Bass:
    nc = bacc.Bacc()
    x_h = nc.dram_tensor("x_h", [BPC, HIGH, N], BF16, kind="ExternalInput")
    x_l = nc.dram_tensor("x_l", [BPC, LOW, N], BF16, kind="ExternalInput")
    thw = nc.dram_tensor("thw", [P, KH, LOW], BF16, kind="ExternalInput")
    thw8 = nc.dram_tensor("thw8", [P, NG8, 2, LOW], FP8, kind="ExternalInput")
    xh8 = nc.dram_tensor("xh8", [BPC, P, NG8, 2, N], FP8, kind="ExternalInput")
    phw = nc.dram_tensor("phw", [P, KL, LOW], BF16, kind="ExternalInput")
    gw = nc.dram_tensor("gw", [P, KL, LOW], BF16, kind="ExternalInput")
    ww = nc.dram_tensor("ww", [P, KL, HIGH], BF16, kind="ExternalInput")
    thpb = nc.dram_tensor("thpb", [1, 2 * LOW], BF16, kind="ExternalInput")
    gbnt = nc.dram_tensor("gbnt", [P, KL + KH], F32, kind="ExternalInput")
    out = nc.dram_tensor("out", [BPC, HIGH, N], BF16, kind="ExternalOutput")
    thp_dram = nc.dram_tensor("thp_dram", [64, MN, LOW], BF16, kind="Internal")

    with tile.TileContext(nc) as tc:
        with (
            tc.tile_pool(name="consts", bufs=1) as cpool,
            tc.tile_pool(name="xh", bufs=KH) as xhpool,
            tc.tile_pool(name="xh8", bufs=2) as xh8pool,
            tc.tile_pool(name="xl", bufs=1) as xlpool,
            tc.tile_pool(name="att", bufs=1) as attpool,
            tc.tile_pool(name="mid", bufs=1) as midpool,
            tc.tile_pool(name="stg", bufs=3) as stgpool,
            tc.tile_pool(name="psum", bufs=8, space="PSUM") as pspool,
        ):
            # PE warmup: the p-state ramp (0.65/1.2 GHz for the first ~3us of
            # PE activity) burns on throwaway matmuls while the first DMAs
            # land, so real matmuls start at the full 2.4 GHz clock
            wu = cpool.tile([P, 64], BF16, tag="wu")
            nc.gpsimd.memset(wu[:], 0.0)
            wps = pspool.tile([P, 512], F32, tag="ps", name="wps")
            for i in range(40):
                nc.tensor.matmul(
                    wps[:64, :64], wu[:], wu[:], start=True, stop=True
                )

            # A2's inputs go first so phi can start ASAP.  Each dma_start
            # costs ~1.2us serialized on its issuing queue, so the three
            # first-dependency transfers are spread across SP, Pool (SWDGE)
            # and ACT so their issue latencies overlap.
            phw_sb = cpool.tile([P, KL, LOW], BF16, tag="phw")
            nc.sync.dma_start(phw_sb[:, :1], phw[:, :1])
            nc.sync.dma_start(phw_sb[:, 1:2], phw[:, 1:2])
            nc.gpsimd.dma_start(phw_sb[:, 2:], phw[:, 2:])
            xl0_sb = xlpool.tile([P, KL, N], BF16, tag="xl")
            xl0_r = x_l[0].rearrange("(ko p) n -> p ko n", p=P)
            # m=0 sub-chunk first so A2's first group starts on ~100KB
            nc.scalar.dma_start(xl0_sb[:, :, :P], xl0_r[:, :, :P])
            nc.scalar.dma_start(xl0_sb[:, :, P:NW], xl0_r[:, :, P:NW])
            for nn in range(1, NSPLIT):
                nc.scalar.dma_start(
                    xl0_sb[:, :, ts(nn, NW)], xl0_r[:, :, ts(nn, NW)]
                )
            thpb_sb = cpool.tile([P, 2 * LOW], BF16, tag="thpb")
            nc.sync.dma_start(thpb_sb[:], thpb[:].to_broadcast((P, 2 * LOW)))
            thb_sb = thpb_sb[:, :LOW]
            phb_sb = thpb_sb[:, LOW:]
            # theta fp8 pair tensors early — the n-half1 partial DR groups
            # interleaved into A2 consume them from ~6us; xh8 split per
            # pair-group so g=0 quadrants can start on the first half
            thw8_sb = cpool.tile([P, NG8, 2, LOW], FP8, tag="thw8")
            nc.sync.dma_start(thw8_sb[:], thw8[:])
            xh8_sb = xh8pool.tile([P, NG8, 2, N], FP8, tag="xh8")
            nc.sync.dma_start(xh8_sb[:, 0], xh8[0, :, 0])
            nc.sync.dma_start(xh8_sb[:, 1], xh8[0, :, 1])
            gbnt_sb = cpool.tile([P, KL + KH], F32, tag="gbnt")
            nc.sync.dma_start(gbnt_sb[:], gbnt[:])
            gb_sb = gbnt_sb[:, :KL]
            bnt_sb = gbnt_sb[:, KL:]
            # bf16 theta inputs in A1's k-major consumption order (k=4..15);
            # the stream is spread across all three issue queues (thw
            # quarters on Pool, xh chunks alternating SP/ACT) so it keeps
            # ahead of the fp8-accelerated A1 consumption; xh chunks 0..3
            # (only needed for C's residual) go last
            thw_sb = cpool.tile([P, KH, LOW], BF16, tag="thw")
            xh_t: list = [None] * KH
            for q in range(1, 4):
                nc.gpsimd.dma_start(
                    thw_sb[:, ts(q, KH // 4), :], thw[:, ts(q, KH // 4), :]
                )
                for k in range(q * 4, q * 4 + 4):
                    t_ = xhpool.tile([P, N], BF16, tag="xh", name=f"xh_{k}")
                    eng = nc.sync if k % 2 == 0 else nc.scalar
                    eng.dma_start(t_[:], x_h[0, ts(k, P), :])
                    xh_t[k] = t_
            for k in range(KB0):
                t_ = xhpool.tile([P, N], BF16, tag="xh", name=f"xh_{k}")
                eng = nc.sync if k % 2 == 0 else nc.scalar
                eng.dma_start(t_[:], x_h[0, ts(k, P), :])
                xh_t[k] = t_
            gw_sb = cpool.tile([P, KL, LOW], BF16, tag="gw")
            nc.scalar.dma_start(gw_sb[:], gw[:])
            ww_sb = cpool.tile([P, KL, HIGH], BF16, tag="ww")
            for k in range(KL):
                nc.sync.dma_start(ww_sb[:, k], ww[:, k])

            for b in range(BPC):
                if b > 0:
                    xl_sb = xl_next
                    xh_t = xh_next
                    xh8_sb = xh8_next
                else:
                    xl_sb = xl0_sb

                # phi_xT [n, d] (phase A2), with theta's n-half1 fp8 partials
                # (tiny DR groups) interleaved so their PSUM allocations ride
                # A2's progressive bank frees
                ph_sb = midpool.tile([P, MN, LOW], BF16, tag="ph")
                th_sb = midpool.tile([P, MN, LOW], BF16, tag="th")
                thp_sb = midpool.tile([64, MN, LOW], BF16, tag="thp")
                thp2_sb = midpool.tile([P, MN, LOW], BF16, tag="thp2")

                def a1_dr(ps_m, m, half, start, stop):
                    for g in range(NG8):
                        for ch in range(2):
                            nc.tensor.matmul(
                                ps_m[0:64, ts(ch, 256)],
                                xh8_sb[:, g, :, m * P + half * 64 : m * P + half * 64 + 64],
                                thw8_sb[:, g, :, ts(ch, 256)],
                                start=start and g == 0 and ch == 0,
                                stop=stop and g == NG8 - 1 and ch == 1,
                                perf_mode=DR,
                                skip_group_check=True,
                            )

                def a1_partial(mp):
                    psp = pspool.tile([P, 512], F32, tag="ps", name=f"ps_p_{mp}")
                    a1_dr(psp, mp, 1, True, True)
                    nc.vector.scalar_tensor_tensor(
                        thp_sb[:, mp, :], psp[0:64, :], 1.0 / TH_SCALE,
                        thb_sb[0:64, :], MULT, ADD,
                    )

                for m in range(MN):
                    ps = pspool.tile([P, 512], F32, tag="ps")
                    for k in range(KL):
                        nc.tensor.matmul(
                            ps[:],
                            xl_sb[:, k, ts(m, P)],
                            phw_sb[:, k, :],
                            start=(k == 0),
                            stop=(k == KL - 1),
                        )
                    nc.vector.tensor_tensor(ph_sb[:, m, :], ps[:], phb_sb[:], ADD)
                    # partials deferred 4 groups so their fp8 inputs land
                    if m >= 4:
                        a1_partial(m - 4)
                for mp in range(MN - 4, MN):
                    a1_partial(mp)
                nc.sync.dma_start(thp_dram[:], thp_sb[:])
                nc.sync.dma_start(thp2_sb[64:128], thp_dram[:])

                # theta_xT [n, c] (phase A1).  Channels 0..255 run as fp8
                # DoubleRow (2x PE rate); DR outputs can only land on PSUM
                # partitions 0..63.  Each main group opens with its n-half0
                # DR quadrants (ch0 start=True arms the 2KB zero region,
                # auto-zeroing partitions 64..127 for the bf16 k=2 matmul),
                # then accumulates bf16 k-major over 8 banks so chunk k is
                # consumed right after it lands; m=8 runs m-major.  The
                # n-half1 partials were computed during A2 and remapped to
                # partitions 64..127; the drains combine them.
                def a1_drain(ps_m, m):
                    nc.vector.scalar_tensor_tensor(
                        th_sb[0:64, m, :], ps_m[0:64, :], 1.0 / TH_SCALE,
                        thb_sb[0:64, :], MULT, ADD,
                    )
                    nc.vector.scalar_tensor_tensor(
                        th_sb[64:128, m, :], ps_m[64:128, :], 1.0 / TH_SCALE,
                        thp2_sb[64:128, m, :], MULT, ADD,
                    )

                ps_a1 = [
                    pspool.tile([P, 512], F32, tag="ps", name=f"ps_a1_{m}")
                    for m in range(6)
                ]
                for m in range(6):
                    # k=2 opens the group full-width (start zeroes the whole
                    # bank incl. partitions 64..127), then the DR quadrants
                    # accumulate onto partitions 0..63
                    nc.tensor.matmul(
                        ps_a1[m][:],
                        xh_t[KB0][:, ts(m, P)],
                        thw_sb[:, KB0, :],
                        start=True,
                        stop=False,
                        skip_group_check=True,
                    )
                    a1_dr(ps_a1[m], m, 0, False, False)
                for k in range(KB0 + 1, KH):
                    for m in range(6):
                        nc.tensor.matmul(
                            ps_a1[m][:],
                            xh_t[k][:, ts(m, P)],
                            thw_sb[:, k, :],
                            start=False,
                            stop=(k == KH - 1),
                            skip_group_check=True,
                        )
                for m in range(6):
                    a1_drain(ps_a1[m], m)
                # m=6,7 run in the spare banks, covering the drain latency;
                # m=8 finishes m-major
                for m in (6, 7, 8):
                    ps = pspool.tile([P, 512], F32, tag="ps", name=f"ps_t_{m}")
                    nc.tensor.matmul(
                        ps[:], xh_t[KB0][:, ts(m, P)], thw_sb[:, KB0, :],
                        start=True, stop=False, skip_group_check=True,
                    )
                    a1_dr(ps, m, 0, False, False)
                    for k in range(KB0 + 1, KH):
                        nc.tensor.matmul(
                            ps[:],
                            xh_t[k][:, ts(m, P)],
                            thw_sb[:, k, :],
                            start=False,
                            stop=(k == KH - 1),
                            skip_group_check=True,
                        )
                    a1_drain(ps, m)

                # g_x [d, n] (phase A3) — sits between theta's drain burst
                # and B1 so the th drains overlap PE work
                g_sb = midpool.tile([P, KL, N], BF16, tag="g")
                for md in range(KL):
                    for nn in range(NSPLIT):
                        ps = pspool.tile([P, 512], F32, tag="ps")
                        for k in range(KL):
                            nc.tensor.matmul(
                                ps[:, :NW],
                                gw_sb[:, k, ts(md, P)],
                                xl_sb[:, k, ts(nn, NW)],
                                start=(k == 0),
                                stop=(k == KL - 1),
                            )
                        nc.scalar.activation(
                            g_sb[:, md, ts(nn, NW)],
                            ps[:, :NW],
                            AF.Identity,
                            bias=gb_sb[:, md : md + 1],
                        )

                # batch b+1 x_l prefetch: WAR on this batch's A2/A3 reads,
                # issued from the ACT queue (behind A3's drains)
                if b + 1 < BPC:
                    xl_next = xlpool.tile([P, KL, N], BF16, tag="xl")
                    xl1_r = x_l[b + 1].rearrange("(ko p) n -> p ko n", p=P)
                    for nn in range(NSPLIT):
                        nc.scalar.dma_start(
                            xl_next[:, :, ts(nn, NW)], xl1_r[:, :, ts(nn, NW)]
                        )

                # attT [d, c] = energy^T/512 (phase B1)
                att_sb = attpool.tile([P, KL, LOW], BF16, tag="att")
                for md in range(KL):
                    ps = pspool.tile([P, 512], F32, tag="ps")
                    for k in range(MN):
                        nc.tensor.matmul(
                            ps[:],
                            ph_sb[:, k, ts(md, P)],
                            th_sb[:, k, :],
                            start=(k == 0),
                            stop=(k == MN - 1),
                        )
                    nc.scalar.activation(att_sb[:, md, :], ps[:], AF.Identity)

                # y [c, n] (phase B2); y shares the theta_xT slot
                y_sb = midpool.tile([P, KL, N], BF16, tag="th")
                for mc in range(KL):
                    for nn in range(NSPLIT):
                        ps = pspool.tile([P, 512], F32, tag="ps")
                        for k in range(KL):
                            nc.tensor.matmul(
                                ps[:, :NW],
                                att_sb[:, k, ts(mc, P)],
                                g_sb[:, k, ts(nn, NW)],
                                start=(k == 0),
                                stop=(k == KL - 1),
                            )
                        nc.scalar.activation(
                            y_sb[:, mc, ts(nn, NW)], ps[:, :NW], AF.Identity
                        )

                # w_y + BN + residual (phase C); output staged per mo stripe
                # and written as one DMA; batch b+1 x_h chunk prefetch issues
                # from ACT right after chunk mo's last read
                for mo in range(KH):
                    xt = xh_t[mo]
                    stg = stgpool.tile([P, N], BF16, tag="stg")
                    last = b == BPC - 1 and mo == KH - 1
                    for nn in range(NSPLIT):
                        ps = pspool.tile([P, 512], F32, tag="ps")
                        for k in range(KL):
                            nc.tensor.matmul(
                                ps[:, :NW],
                                ww_sb[:, k, ts(mo, P)],
                                y_sb[:, k, ts(nn, NW)],
                                start=(k == 0),
                                stop=(k == KL - 1),
                            )
                        nc.vector.scalar_tensor_tensor(
                            stg[:, ts(nn, NW)],
                            ps[:, :NW],
                            bnt_sb[:, mo : mo + 1],
                            xt[:, ts(nn, NW)],
                            ADD,
                            ADD,
                        )
                        # the very last stripe writes out in two DMAs so the
                        # final transfer after the last matmul is small
                        if last and nn == NSPLIT - 2:
                            nc.sync.dma_start(
                                out[b, ts(mo, P), : 2 * NW], stg[:, : 2 * NW]
                            )
                        elif last and nn == NSPLIT - 1:
                            nc.sync.dma_start(
                                out[b, ts(mo, P), 2 * NW :], stg[:, 2 * NW :]
                            )
                    if not last:
                        nc.sync.dma_start(out[b, ts(mo, P), :], stg[:])
                    if b + 1 < BPC:
                        if mo == 0:
                            xh_next = [None] * KH
                            xh8_next = xh8pool.tile([P, NG8, 2, N], FP8, tag="xh8")
                            nc.scalar.dma_start(xh8_next[:], xh8[b + 1])
                        t_ = xhpool.tile([P, N], BF16, tag="xh")
                        nc.scalar.dma_start(t_[:], x_h[b + 1, ts(mo, P), :])
                        xh_next[mo] = t_
    nc.compile()
    return nc


_CACHE: dict = {}


def _get_module() -> bass.Bass:
    if "nc" not in _CACHE:
        _CACHE["nc"] = _build_module()
    return _CACHE["nc"]


def _prep_maps(inputs: dict) -> list[dict]:
    import ml_dtypes

    BF = ml_dtypes.bfloat16
    f = lambda a: np.ascontiguousarray(np.asarray(a, dtype=np.float32))
    bf = lambda a: np.ascontiguousarray(np.asarray(a, dtype=np.float32).astype(BF))
    x_h = bf(inputs["x_h"]).reshape(B, HIGH, N)
    x_l = bf(inputs["x_l"]).reshape(B, LOW, N)
    theta_w = f(inputs["theta_w"])
    phi_w = f(inputs["phi_w"])
    g_w = f(inputs["g_w"])
    w_w = f(inputs["w_w"])

    # bf16 theta weights pre-scaled by 2^11 to match the fp8 partial's scale
    thw_h = (theta_w.T * np.float32(TH_SCALE)).reshape(KH, P, LOW) \
        .transpose(1, 0, 2).astype(BF)
    # fp8 pair tensors for the DoubleRow channels (0..255): pair index i
    # holds channels i*128+p
    F8 = ml_dtypes.float8_e4m3
    clip8 = lambda a: np.clip(a, -224.0, 224.0).astype(F8)
    # thw8[p, g, i, c] = theta_w[c, g*256 + i*128 + p] * 256
    thw8_h = np.ascontiguousarray(
        clip8(
            (theta_w[:, :KF8] * np.float32(THW8_SCALE)).T
            .reshape(NG8, 2, P, LOW).transpose(2, 0, 1, 3)
        )
    )
    # xh8[b, p, g, i, n] = x_h[b, g*256 + i*128 + p, n] * 8
    x_h32 = f(inputs["x_h"]).reshape(B, HIGH, N)
    xh8_h = np.ascontiguousarray(
        clip8(
            (x_h32[:, :KF8, :] * np.float32(XH8_SCALE))
            .reshape(B, NG8, 2, P, N).transpose(0, 3, 1, 2, 4)
        )
    )
    phw_h = (phi_w.T / np.float32(LOW)).reshape(KL, P, LOW).transpose(1, 0, 2).astype(BF)
    gw_h = g_w.T.reshape(KL, P, LOW).transpose(1, 0, 2).astype(BF)
    s = f(inputs["bn_gamma"]) / np.sqrt(f(inputs["bn_var"]) + np.float32(BN_EPS))
    # BN scale folded into the w conv weights; only the shift remains on-device
    ww_h = (w_w * s[:, None]).astype(np.float32).T.reshape(KL, P, HIGH) \
        .transpose(1, 0, 2).astype(BF)

    thpb_h = np.concatenate(
        [f(inputs["theta_b"]), f(inputs["phi_b"]) / np.float32(LOW)]
    ).reshape(1, 2 * LOW).astype(BF)
    gb_h = np.ascontiguousarray(f(inputs["g_b"]).reshape(KL, P).T)
    t = (f(inputs["w_b"]) - f(inputs["bn_mean"])) * s + f(inputs["bn_beta"])
    bnt_h = np.ascontiguousarray(t.astype(np.float32).reshape(KH, P).T)
    gbnt_h = np.ascontiguousarray(np.concatenate([gb_h, bnt_h], axis=1))

    shared = dict(
        thw=np.ascontiguousarray(thw_h),
        thw8=thw8_h,
        phw=np.ascontiguousarray(phw_h),
        gw=np.ascontiguousarray(gw_h),
        ww=np.ascontiguousarray(ww_h),
        thpb=thpb_h,
        gbnt=gbnt_h,
    )
    maps = []
    for c in range(NCORES):
        m = dict(shared)
        m["x_h"] = np.ascontiguousarray(x_h[c * BPC : (c + 1) * BPC])
        m["xh8"] = np.ascontiguousarray(xh8_h[c * BPC : (c + 1) * BPC])
        m["x_l"] = np.ascontiguousarray(x_l[c * BPC : (c + 1) * BPC])
        maps.append(m)
    return maps


def _run(inputs: dict, **kwargs):
    from concourse.bass_utils import run_bass_kernel_spmd

    nc = _get_module()
    in_maps = _prep_maps(inputs)
    res = run_bass_kernel_spmd(nc, in_maps, core_ids=list(range(NCORES)), **kwargs)
    parts = [np.asarray(r["out"], dtype=np.float32) for r in res.results]
    full = np.concatenate(parts, axis=0).reshape(B, HIGH, H, W)
    return full, res


def kernel(**inputs) -> np.ndarray:
    full, _ = _run(inputs)
    return full
